# revision 21
# baseline (speedup 1.0000x reference)
"""CrossAttention Trainium2 kernel.

Reference computation (per batch b):
  q = x @ Wq; k = ctx @ Wk; v = ctx @ Wv   (multi-head, H=8, DH=64)
  out = softmax(q k^T / sqrt(DH)) v @ Wo + bo, rows >= seq_len zeroed.

Strategy: only rows < seq_len are computed ("ragged"); valid query tiles
(512 rows) are distributed across the 8 cores with a statically uniform
per-core structure: every core runs CAP query tiles, the first CAP_A of
which read KV slot A and the rest KV slot B. Which batch each slot holds
is per-core DATA (context tensors), so one SPMD program serves all cores.

Per query tile (Lt = 512 rows):
  xT [DQ, Lt] -> qT = Wq^T xT [INNER, Lt]            (f32r matmuls)
  kT = Wk^T ctxT [INNER, S] per slot                 (f32r)
  v_aug [S, 8, 65] = ctx @ Wv + ones col per head    (bf16)
  scoresT_h [S, Lt] = kT_h^T qT_h  (per head, K=DH)  (f32r)
  expT = exp(scoresT) -> bf16  (no max subtraction: logits ~ N(0,1))
  AV reoriented: av[q, d]: out [128q, 4h, 65] psum, lhsT = expT chunk
    (stationary), rhs = v_aug_h [s, 65] moving, N=65 bf16 (1 cyc/row) --
    2.3x fewer PE cycles than the avT orientation, and the softmax
    denominator (col 64) lands on the q-PARTITION axis where
    normalization is cheap:
  norm: rcp[q, h] = 1/pav[:, :, 64];  avn = pav[:, :, 0:64] * rcp  (DVE)
  transpose avn -> avT [INNER, Lt] via PE is_transpose matmuls (bf16)
  out [Lt, DQ] = avT^T @ Wo + bob                     (bf16 matmuls)
Emission interleaves next-tile scores with the current tile's AV /
transpose / output projection so the in-order PE and ACT streams both
stay busy; slot-B KV matmuls are spread across the two tiles preceding
the slot boundary.
"""

import math
import sys

sys.path.insert(0, "/opt/trn_rl_repo")

import numpy as np

B, L, S = 8, 8192, 512
DQ, DC = 256, 768
H, DH = 8, 64
INNER = H * DH
TL = 512          # query rows per tile
N_CORES = 8
WVN = H * 65      # 520: v augmented with a ones col per head


def _plan(nt):
    """Choose (CAP_A, CAP_B) and per-core pieces. Returns
    (cap_a, cap_b, cores) where cores is a list of 8 entries
    [(batch_a, tile0_a), (batch_b, tile0_b)] (batch -1 = padding)."""
    best = None
    lo = max(1, math.ceil(sum(nt) / N_CORES))
    for cap in range(lo, max(max(nt), lo) + 9):
        # NB=1: whole batches in CAP-size pieces
        if sum(math.ceil(n / cap) for n in nt) <= N_CORES:
            cost = cap * 18 + 12
            if best is None or cost < best[0]:
                best = (cost, cap, 0, None)
        # NB=2 split
        for a in range(cap - 1, 0, -1):
            b = cap - a
            opts = []
            for n in nt:
                o = []
                for ka in range(0, N_CORES + 1):
                    rem = n - a * ka
                    kb = max(0, math.ceil(rem / b))
                    if kb <= N_CORES:
                        o.append((ka, kb))
                opts.append(o)

            found = None

            def dfs(i, ta, tb, acc):
                nonlocal found
                if found is not None:
                    return
                if i == len(opts):
                    found = list(acc)
                    return
                for ka, kb in opts[i]:
                    if ta + ka <= N_CORES and tb + kb <= N_CORES:
                        acc.append((ka, kb))
                        dfs(i + 1, ta + ka, tb + kb, acc)
                        acc.pop()
                        if found is not None:
                            return

            dfs(0, 0, 0, [])
            if found is not None:
                cost = cap * 18 + 24
                if best is None or cost < best[0]:
                    best = (cost, cap, 1, (a, b, found))
                break  # larger a preferred; next a adds nothing
    assert best is not None
    _, cap, kind, info = best
    if kind == 0:
        # NB=1: emit as (a=cap, b=0-like) with slot B duplicating slot A
        pieces_a = []
        for bi, n in enumerate(nt):
            for j in range(math.ceil(n / cap)):
                pieces_a.append((bi, j * cap))
        while len(pieces_a) < N_CORES:
            pieces_a.append((-1, 0))
        cores = [[pa, (-1, 0)] for pa in pieces_a]
        return cap, 0, cores
    a, bsz, ks = info
    pieces_a, pieces_b = [], []
    for bi, n in enumerate(nt):
        ka, kb = ks[bi]
        t = 0
        for _ in range(ka):
            pieces_a.append((bi, t))
            t += a
        for _ in range(kb):
            pieces_b.append((bi, t))
            t += bsz
    while len(pieces_a) < N_CORES:
        pieces_a.append((-1, 0))
    while len(pieces_b) < N_CORES:
        pieces_b.append((-1, 0))
    cores = [[pieces_a[i], pieces_b[i]] for i in range(N_CORES)]
    return a, bsz, cores


_PROG_CACHE = {}


def _build_program(cap_a, cap_b):
    import concourse.mybir as mybir
    import concourse.tile as tile
    from concourse import bacc

    f32 = mybir.dt.float32
    f32r = mybir.dt.float32r
    bf16 = mybir.dt.bfloat16
    CAP = cap_a + cap_b
    NSLOT = 2 if cap_b > 0 else 1

    nc = bacc.Bacc("TRN2", target_bir_lowering=False, debug=False,
                   num_devices=N_CORES)
    xT = nc.declare_dram_parameter("xT", [DQ, CAP * TL], bf16, isOutput=False)
    ctxs = [nc.declare_dram_parameter(f"ctx{s}", [DC, S], bf16, isOutput=False)
            for s in range(NSLOT)]
    wq = nc.declare_dram_parameter("wq", [DQ, INNER], bf16, isOutput=False)
    wk = nc.declare_dram_parameter("wk", [DC, INNER], bf16, isOutput=False)
    wv = nc.declare_dram_parameter("wv", [DC, INNER], bf16, isOutput=False)
    wo = nc.declare_dram_parameter("wo", [INNER, DQ], bf16, isOutput=False)
    bob = nc.declare_dram_parameter("bob", [128, DQ], f32, isOutput=False)
    ident = nc.declare_dram_parameter("ident", [128, 128], bf16, isOutput=False)
    y = nc.declare_dram_parameter("y", [CAP * TL, DQ], f32, isOutput=True)

    with tile.TileContext(nc) as tc:
        with (
            tc.tile_pool(name="wpool", bufs=1) as wpool,
            tc.tile_pool(name="kvpool", bufs=1) as kvpool,
            tc.tile_pool(name="ctxpool", bufs=1) as ctxpool,
            tc.tile_pool(name="mpool", bufs=4) as mpool,
            tc.tile_pool(name="qpool", bufs=3) as qpool,
            tc.tile_pool(name="epool", bufs=14) as epool,
            tc.tile_pool(name="anpool", bufs=12) as anpool,
            tc.tile_pool(name="ovpool", bufs=6) as ovpool,
            tc.tile_pool(name="spool", bufs=4) as spool,
            tc.tile_pool(name="ypool", bufs=4) as ypool,
            tc.tile_pool(name="ps_big", bufs=2, space="PSUM") as ps_big,
            tc.tile_pool(name="ps_sc", bufs=2, space="PSUM") as ps_sc,
            tc.tile_pool(name="ps_av", bufs=2, space="PSUM") as ps_av,
        ):
            # ---- load weights. The head is DMA-latency critical: spread
            # the issue cost across idle engine queues (SP / ACT / DVE all
            # have their own DGE rings) so the shared DMA engines are the
            # only serializer before the first score matmul.
            wq_sb = [wpool.tile([128, INNER], bf16, tag=f"wq{i}", name=f"wq{i}") for i in range(2)]
            for i in range(2):
                nc.sync.dma_start(wq_sb[i][:], wq[i * 128:(i + 1) * 128, :])
            pre_x = {}
            xt_t = [mpool.tile([128, TL], bf16, tag=f"x{kc}", name=f"x{kc}")
                    for kc in range(2)]
            for kc in range(2):
                nc.sync.dma_start(xt_t[kc][:], xT[kc * 128:(kc + 1) * 128, 0:TL])
            pre_x[0] = xt_t
            wk_sb = [wpool.tile([128, INNER], bf16, tag=f"wk{i}", name=f"wk{i}") for i in range(6)]
            ctx_tiles = {}
            for s in range(NSLOT):
                ctx_tiles[s] = [ctxpool.tile([128, S], bf16, tag=f"ctx{s}_{i}", name=f"ctx{s}_{i}")
                                for i in range(6)]
            for i in range(6):
                nc.scalar.dma_start(wk_sb[i][:], wk[i * 128:(i + 1) * 128, :])
                nc.sync.dma_start(ctx_tiles[0][i][:], ctxs[0][i * 128:(i + 1) * 128, :])
            if CAP > 1:
                xt_t = [mpool.tile([128, TL], bf16, tag=f"x{kc}", name=f"x{kc}")
                        for kc in range(2)]
                for kc in range(2):
                    nc.sync.dma_start(xt_t[kc][:], xT[kc * 128:(kc + 1) * 128, TL:2 * TL])
                pre_x[1] = xt_t
            wv_sb = [wpool.tile([128, INNER], bf16, tag=f"wv{i}", name=f"wv{i}") for i in range(6)]
            for i in range(6):
                nc.sync.dma_start(wv_sb[i][:], wv[i * 128:(i + 1) * 128, :])
            wo_sb = [wpool.tile([128, DQ], bf16, tag=f"wo{i}", name=f"wo{i}") for i in range(4)]
            for i in range(4):
                nc.sync.dma_start(wo_sb[i][:], wo[i * 128:(i + 1) * 128, :])
            bob_sb = wpool.tile([128, DQ], f32, tag="bob", name="bob")
            nc.sync.dma_start(bob_sb[:], bob[:])
            id_sb = wpool.tile([128, 128], bf16, tag="ident", name="ident")
            nc.sync.dma_start(id_sb[:], ident[:])
            for s in range(1, NSLOT):
                for i in range(6):
                    nc.sync.dma_start(ctx_tiles[s][i][:], ctxs[s][i * 128:(i + 1) * 128, :])

            # ---- KV pieces: i in 0..3 -> kT m-chunk i; 4..7 -> v s-chunk i-4
            kT = {}
            vA = {}

            def kv_piece(s, i):
                ctx_sb = ctx_tiles[s]
                if s not in kT:
                    kT[s] = [kvpool.tile([128, S], bf16, tag=f"kT{s}_{m}",
                                         name=f"kT{s}_{m}") for m in range(4)]
                    vA[s] = [kvpool.tile([128, WVN], bf16, tag=f"v{s}_{sc}",
                                         name=f"v{s}_{sc}") for sc in range(4)]
                if i < 4:
                    m = i
                    pk = ps_big.tile([128, S], f32, tag="big", name="big")
                    for kc in range(6):
                        nc.tensor.matmul(
                            pk[:], wk_sb[kc][:, m * 128:(m + 1) * 128],
                            ctx_sb[kc][:], start=(kc == 0), stop=(kc == 5))
                    nc.vector.tensor_copy(kT[s][m][:], pk[:])
                else:
                    sc = i - 4
                    pv = ps_big.tile([128, 512], f32, tag="big", name="big")
                    for kc in range(6):
                        nc.tensor.matmul(
                            pv[:],
                            ctx_sb[kc][:, sc * 128:(sc + 1) * 128],
                            wv_sb[kc][:], start=(kc == 0), stop=(kc == 5))
                    vdst = vA[s][sc][:].rearrange("p (h d) -> p h d", d=65)
                    nc.vector.tensor_copy(
                        vdst[:, :, 0:DH],
                        pv[:].rearrange("p (h d) -> p h d", d=DH))
                    nc.gpsimd.memset(vdst[:, :, DH:65], 1.0)

            # ---- helpers for the main loop ----
            def emit_scores(qT_t, s, h):
                """4 score matmuls + 2 exps for head h; returns 4 expT
                slices [(etile, j)] for s-chunks 0..3."""
                c, half = h // 2, h % 2
                out = []
                for g in range(2):
                    psc = ps_sc.tile([128, 2, TL], f32, tag="sc", name="sc")
                    for j in range(2):
                        sc = g * 2 + j
                        nc.tensor.matmul(
                            psc[:, j, :],
                            kT[s][c][half * 64:(half + 1) * 64,
                                     sc * 128:(sc + 1) * 128],
                            qT_t[c][half * 64:(half + 1) * 64, :],
                            start=True, stop=True)
                    e = epool.tile([128, 2, TL], bf16, tag=f"e{g}", name=f"e{g}")
                    nc.scalar.activation(
                        e[:], psc[:], mybir.ActivationFunctionType.Exp)
                    out.extend([(e, 0), (e, 1)])
                return out

            def emit_av(s, grp, exps, avn_out, qcs=(0, 1, 2, 3)):
                """AV + normalize for heads grp*4..grp*4+3; fills
                avn_out[grp] = list of 4 avn tiles (one per q-chunk)."""
                for qc in qcs:
                    pav = ps_av.tile([128, 4, 65], f32, tag="av", name="av")
                    for hh in range(4):
                        h = grp * 4 + hh
                        for sc in range(4):
                            e, j = exps[h][sc]
                            nc.tensor.matmul(
                                pav[:, hh, :],
                                e[:, j, qc * 128:(qc + 1) * 128],
                                vA[s][sc][:, h * 65:(h + 1) * 65],
                                start=(sc == 0), stop=(sc == 3))
                    rcp = spool.tile([128, 4], f32, tag="rcp", name="rcp")
                    nc.vector.reciprocal(rcp[:], pav[:, :, 64:65])
                    avn = anpool.tile([128, 4, DH], bf16, tag="avn", name="avn")
                    nc.vector.tensor_mul(
                        avn[:], pav[:, :, 0:DH],
                        rcp[:].rearrange("p (h o) -> p h o", o=1)
                        .broadcast_to([128, 4, DH]))
                    avn_out[grp][qc] = avn

            def emit_transp(avn_tiles):
                """Transpose avn [q, inner] -> 4 sbuf tiles [128 i, 512 q]."""
                avT = []
                for ic in range(4):
                    tp = ps_big.tile([128, 512], bf16, tag="big", name="big")
                    grp, pr = ic // 2, ic % 2
                    for qc in range(4):
                        nc.tensor.transpose(
                            tp[:, qc * 128:(qc + 1) * 128],
                            avn_tiles[grp][qc][:, pr * 2:pr * 2 + 2, :],
                            id_sb[:])
                    av_sb = ovpool.tile([128, 512], bf16, tag="avT", name="avT")
                    nc.vector.tensor_copy(av_sb[:], tp[:])
                    avT.append(av_sb)
                return avT

            def emit_oproj(tt, avT):
                for lsub in range(4):
                    po = ps_big.tile([128, DQ], f32, tag="big", name="big")
                    for kc in range(4):
                        nc.tensor.matmul(
                            po[:], avT[kc][:, lsub * 128:(lsub + 1) * 128],
                            wo_sb[kc][:], start=(kc == 0), stop=(kc == 3))
                    yt = ypool.tile([128, DQ], f32, tag="y", name="y")
                    nc.vector.tensor_add(yt[:], po[:], bob_sb[:])
                    nc.sync.dma_start(
                        y[tt * TL + lsub * 128: tt * TL + (lsub + 1) * 128, :],
                        yt[:])

            # ---- main loop over query tiles ----
            # pending work carried into the next tile (keeps PE fed while
            # ACT finishes the current tile's exps).
            # kv_sched[t] = [(slot, piece, sc_pos)]: emit KV piece right
            # before score head sc_pos of tile t, so the in-order PE stream
            # never runs a long KV burst that would starve ACT.
            pend = None  # (t, s, exps, avn_tiles)
            kv_sched = {0: [(0, 0, 0), (0, 1, 2), (0, 4, 3), (0, 2, 4),
                            (0, 5, 4), (0, 3, 5), (0, 6, 5), (0, 7, 5)]}
            if NSLOT > 1:
                free_tiles = list(range(1, cap_a))
                if len(free_tiles) >= 4:
                    sched = []
                    ft = free_tiles[-4:]
                    for j in range(8):
                        sched.append((1, j, 2 if j % 2 == 0 else 5))
                    kv_sched.update({
                        ft[0]: [sched[0], sched[1]],
                        ft[1]: [sched[2], sched[3]],
                        ft[2]: [sched[4], sched[5]],
                        ft[3]: [sched[6], sched[7]],
                    })
                elif free_tiles:
                    per = 8 // len(free_tiles) + 1
                    k = 0
                    for ftl in free_tiles:
                        pcs = []
                        for _ in range(per):
                            if k < 8:
                                pcs.append((1, k, 2 + 3 * (len(pcs) % 2)))
                                k += 1
                        kv_sched[ftl] = pcs
                    if k < 8:
                        kv_sched[0] = kv_sched[0] + [
                            (1, i, 5) for i in range(k, 8)]
                else:
                    kv_sched[0] = kv_sched[0] + [(1, i, 5) for i in range(8)]

            for t in range(CAP):
                s = 0 if t < cap_a else 1
                sched_t = kv_sched.get(t, [])

                def kv_at(pos):
                    for sl, i, p in sched_t:
                        if p == pos:
                            kv_piece(sl, i)

                if t in pre_x:
                    xt = pre_x.pop(t)
                else:
                    xt = [mpool.tile([128, TL], bf16, tag=f"x{kc}", name=f"x{kc}") for kc in range(2)]
                    for kc in range(2):
                        nc.sync.dma_start(
                            xt[kc][:], xT[kc * 128:(kc + 1) * 128, t * TL:(t + 1) * TL])
                qT_t = [qpool.tile([128, TL], bf16, tag=f"q{m}", name=f"q{m}") for m in range(4)]
                for m in range(4):
                    pq = ps_big.tile([128, TL], f32, tag="big", name="big")
                    for kc in range(2):
                        nc.tensor.matmul(
                            pq[:], wq_sb[kc][:, m * 128:(m + 1) * 128],
                            xt[kc][:], start=(kc == 0), stop=(kc == 1))
                    nc.vector.tensor_copy(qT_t[m][:], pq[:])

                exps = {}
                avn_tiles = {0: [None] * 4, 1: [None] * 4}
                kv_at(0)
                exps[0] = emit_scores(qT_t, s, 0)
                kv_at(1)
                exps[1] = emit_scores(qT_t, s, 1)
                if pend is not None:
                    pt, ps_, pexps, pavn = pend
                    emit_av(ps_, 1, pexps, pavn, qcs=(0, 1))
                kv_at(2)
                exps[2] = emit_scores(qT_t, s, 2)
                if pend is not None:
                    emit_av(ps_, 1, pexps, pavn, qcs=(2, 3))
                kv_at(3)
                exps[3] = emit_scores(qT_t, s, 3)
                if pend is not None:
                    pavT = emit_transp(pavn)
                kv_at(4)
                exps[4] = emit_scores(qT_t, s, 4)
                kv_at(5)
                exps[5] = emit_scores(qT_t, s, 5)
                if pend is not None:
                    emit_oproj(pt, pavT)
                    pend = None
                emit_av(s, 0, exps, avn_tiles, qcs=(0, 1))
                kv_at(6)
                exps[6] = emit_scores(qT_t, s, 6)
                emit_av(s, 0, exps, avn_tiles, qcs=(2, 3))
                kv_at(7)
                exps[7] = emit_scores(qT_t, s, 7)
                pend = (t, s, exps, avn_tiles)

            # flush the last tile's tail
            pt, ps_, pexps, pavn = pend
            emit_av(ps_, 1, pexps, pavn)
            pavT = emit_transp(pavn)
            emit_oproj(pt, pavT)
    nc.compile()
    return nc


def kernel(x, context, seq_lens, Wq, Wk, Wv, Wo, bo):
    from concourse.bass_utils import run_bass_kernel_spmd
    from ml_dtypes import bfloat16

    x = np.asarray(x, dtype=np.float32)
    context = np.asarray(context, dtype=np.float32)
    seq_lens = np.asarray(seq_lens, dtype=np.int32)
    Wq = np.asarray(Wq, dtype=np.float32)
    Wk = np.asarray(Wk, dtype=np.float32)
    Wv = np.asarray(Wv, dtype=np.float32)
    Wo = np.asarray(Wo, dtype=np.float32)
    bo = np.asarray(bo, dtype=np.float32)

    lens = np.clip(seq_lens, 1, L)
    nt = [int(math.ceil(int(n) / TL)) for n in lens]
    cap_a, cap_b, cores = _plan(nt)
    CAP = cap_a + cap_b
    NSLOT = 2 if cap_b > 0 else 1

    key = (cap_a, cap_b)
    if key not in _PROG_CACHE:
        _PROG_CACHE[key] = _build_program(cap_a, cap_b)
    nc = _PROG_CACHE[key]

    # shared (replicated) weights
    scale = 1.0 / math.sqrt(DH)
    wq_in = (Wq * scale).astype(bfloat16)
    wk_in = np.ascontiguousarray(Wk.astype(bfloat16))
    wv_in = np.ascontiguousarray(Wv.astype(bfloat16))
    wo_in = np.ascontiguousarray(Wo.astype(bfloat16))
    bob_in = np.broadcast_to(bo[None, :], (128, DQ)).copy()
    ident_in = np.eye(128, dtype=bfloat16)

    in_maps = []
    for core in range(N_CORES):
        xt_core = np.zeros((CAP * TL, DQ), dtype=np.float32)
        m = {}
        for sidx in range(NSLOT):
            bi, t0 = cores[core][sidx]
            npieces = cap_a if sidx == 0 else cap_b
            if bi >= 0:
                r0 = t0 * TL
                r1 = min(r0 + npieces * TL, L)
                if r1 > r0:
                    off = sidx * cap_a * TL
                    xt_core[off:off + (r1 - r0)] = x[bi, r0:r1]
                cb = context[bi]
            else:
                cb = context[0]
            m[f"ctx{sidx}"] = np.ascontiguousarray(cb.T.astype(bfloat16))
        m["xT"] = np.ascontiguousarray(xt_core.T.astype(bfloat16))
        m["wq"] = wq_in
        m["wk"] = wk_in
        m["wv"] = wv_in
        m["wo"] = wo_in
        m["bob"] = bob_in
        m["ident"] = ident_in
        in_maps.append(m)

    res = run_bass_kernel_spmd(nc, in_maps, list(range(N_CORES)))

    out = np.zeros((B, L, DQ), dtype=np.float32)
    for core in range(N_CORES):
        yc = res.results[core]["y"]
        for sidx in range(NSLOT):
            bi, t0 = cores[core][sidx]
            if bi < 0:
                continue
            npieces = cap_a if sidx == 0 else cap_b
            r0 = t0 * TL
            r1 = min(r0 + npieces * TL, int(lens[bi]))
            if r1 > r0:
                off = sidx * cap_a * TL
                out[bi, r0:r1] = yc[off:off + (r1 - r0)]
    return out


# revision 23
# speedup vs baseline: 1.0192x; 1.0192x over previous
"""CrossAttention Trainium2 kernel.

Reference computation (per batch b):
  q = x @ Wq; k = ctx @ Wk; v = ctx @ Wv   (multi-head, H=8, DH=64)
  out = softmax(q k^T / sqrt(DH)) v @ Wo + bo, rows >= seq_len zeroed.

Strategy: only rows < seq_len are computed ("ragged"); valid query tiles
(512 rows) are distributed across the 8 cores with a statically uniform
per-core structure: every core runs CAP query tiles, the first CAP_A of
which read KV slot A and the rest KV slot B. Which batch each slot holds
is per-core DATA (context tensors), so one SPMD program serves all cores.

Per query tile (Lt = 512 rows):
  xT [DQ, Lt] -> qT = Wq^T xT [INNER, Lt]            (f32r matmuls)
  kT = Wk^T ctxT [INNER, S] per slot                 (f32r)
  v_aug [S, 8, 65] = ctx @ Wv + ones col per head    (bf16)
  scoresT_h [S, Lt] = kT_h^T qT_h  (per head, K=DH)  (f32r)
  expT = exp(scoresT) -> bf16  (no max subtraction: logits ~ N(0,1))
  AV reoriented: av[q, d]: out [128q, 4h, 65] psum, lhsT = expT chunk
    (stationary), rhs = v_aug_h [s, 65] moving, N=65 bf16 (1 cyc/row) --
    2.3x fewer PE cycles than the avT orientation, and the softmax
    denominator (col 64) lands on the q-PARTITION axis where
    normalization is cheap:
  norm: rcp[q, h] = 1/pav[:, :, 64];  avn = pav[:, :, 0:64] * rcp  (DVE)
  transpose avn -> avT [INNER, Lt] via PE is_transpose matmuls (bf16)
  out [Lt, DQ] = avT^T @ Wo + bob                     (bf16 matmuls)
Emission interleaves next-tile scores with the current tile's AV /
transpose / output projection so the in-order PE and ACT streams both
stay busy; slot-B KV matmuls are spread across the two tiles preceding
the slot boundary.
"""

import math
import sys

sys.path.insert(0, "/opt/trn_rl_repo")

import numpy as np

B, L, S = 8, 8192, 512
DQ, DC = 256, 768
H, DH = 8, 64
INNER = H * DH
TL = 512          # query rows per tile
N_CORES = 8
WVN = H * 65      # 520: v augmented with a ones col per head


def _plan(nt):
    """Choose (CAP_A, CAP_B) and per-core pieces. Returns
    (cap_a, cap_b, cores) where cores is a list of 8 entries
    [(batch_a, tile0_a), (batch_b, tile0_b)] (batch -1 = padding)."""
    best = None
    lo = max(1, math.ceil(sum(nt) / N_CORES))
    for cap in range(lo, max(max(nt), lo) + 9):
        # NB=1: whole batches in CAP-size pieces
        if sum(math.ceil(n / cap) for n in nt) <= N_CORES:
            cost = cap * 18 + 12
            if best is None or cost < best[0]:
                best = (cost, cap, 0, None)
        # NB=2 split
        for a in range(cap - 1, 0, -1):
            b = cap - a
            opts = []
            for n in nt:
                o = []
                for ka in range(0, N_CORES + 1):
                    rem = n - a * ka
                    kb = max(0, math.ceil(rem / b))
                    if kb <= N_CORES:
                        o.append((ka, kb))
                opts.append(o)

            found = None

            def dfs(i, ta, tb, acc):
                nonlocal found
                if found is not None:
                    return
                if i == len(opts):
                    found = list(acc)
                    return
                for ka, kb in opts[i]:
                    if ta + ka <= N_CORES and tb + kb <= N_CORES:
                        acc.append((ka, kb))
                        dfs(i + 1, ta + ka, tb + kb, acc)
                        acc.pop()
                        if found is not None:
                            return

            dfs(0, 0, 0, [])
            if found is not None:
                cost = cap * 18 + 24
                if best is None or cost < best[0]:
                    best = (cost, cap, 1, (a, b, found))
                break  # larger a preferred; next a adds nothing
    assert best is not None
    _, cap, kind, info = best
    if kind == 0:
        # NB=1: emit as (a=cap, b=0-like) with slot B duplicating slot A
        pieces_a = []
        for bi, n in enumerate(nt):
            for j in range(math.ceil(n / cap)):
                pieces_a.append((bi, j * cap))
        while len(pieces_a) < N_CORES:
            pieces_a.append((-1, 0))
        cores = [[pa, (-1, 0)] for pa in pieces_a]
        return cap, 0, cores
    a, bsz, ks = info
    pieces_a, pieces_b = [], []
    for bi, n in enumerate(nt):
        ka, kb = ks[bi]
        t = 0
        for _ in range(ka):
            pieces_a.append((bi, t))
            t += a
        for _ in range(kb):
            pieces_b.append((bi, t))
            t += bsz
    while len(pieces_a) < N_CORES:
        pieces_a.append((-1, 0))
    while len(pieces_b) < N_CORES:
        pieces_b.append((-1, 0))
    cores = [[pieces_a[i], pieces_b[i]] for i in range(N_CORES)]
    return a, bsz, cores


_PROG_CACHE = {}


def _build_program(cap_a, cap_b):
    import concourse.mybir as mybir
    import concourse.tile as tile
    from concourse import bacc

    f32 = mybir.dt.float32
    f32r = mybir.dt.float32r
    bf16 = mybir.dt.bfloat16
    CAP = cap_a + cap_b
    NSLOT = 2 if cap_b > 0 else 1

    nc = bacc.Bacc("TRN2", target_bir_lowering=False, debug=False,
                   num_devices=N_CORES)
    xT = nc.declare_dram_parameter("xT", [128, CAP, 2, TL], bf16, isOutput=False)
    ctxs = [nc.declare_dram_parameter(f"ctx{s}", [128, 6, S], bf16, isOutput=False)
            for s in range(NSLOT)]
    wq = nc.declare_dram_parameter("wq", [128, 2, INNER], bf16, isOutput=False)
    wk = nc.declare_dram_parameter("wk", [128, 6, INNER], bf16, isOutput=False)
    wv = nc.declare_dram_parameter("wv", [128, 6, INNER], bf16, isOutput=False)
    wo = nc.declare_dram_parameter("wo", [128, 4, DQ], bf16, isOutput=False)
    bob = nc.declare_dram_parameter("bob", [128, DQ], f32, isOutput=False)
    ident = nc.declare_dram_parameter("ident", [128, 128], bf16, isOutput=False)
    y = nc.declare_dram_parameter("y", [CAP * TL, DQ], f32, isOutput=True)

    with tile.TileContext(nc) as tc:
        with (
            tc.tile_pool(name="wpool", bufs=1) as wpool,
            tc.tile_pool(name="kvpool", bufs=1) as kvpool,
            tc.tile_pool(name="ctxpool", bufs=1) as ctxpool,
            tc.tile_pool(name="mpool", bufs=4) as mpool,
            tc.tile_pool(name="qpool", bufs=3) as qpool,
            tc.tile_pool(name="epool", bufs=14) as epool,
            tc.tile_pool(name="anpool", bufs=12) as anpool,
            tc.tile_pool(name="ovpool", bufs=6) as ovpool,
            tc.tile_pool(name="spool", bufs=4) as spool,
            tc.tile_pool(name="ypool", bufs=4) as ypool,
            tc.tile_pool(name="ps_big", bufs=2, space="PSUM") as ps_big,
            tc.tile_pool(name="ps_sc", bufs=2, space="PSUM") as ps_sc,
            tc.tile_pool(name="ps_av", bufs=2, space="PSUM") as ps_av,
        ):
            # ---- load weights. The head is DMA-latency critical: spread
            # the issue cost across idle engine queues (SP / ACT / DVE all
            # have their own DGE rings) so the shared DMA engines are the
            # only serializer before the first score matmul.
            wq_sb = wpool.tile([128, 2, INNER], bf16, tag="wq", name="wq")
            nc.sync.dma_start(wq_sb[:], wq[:])
            pre_x = {}
            xt_t = mpool.tile([128, 2, TL], bf16, tag="x", name="x")
            nc.sync.dma_start(xt_t[:], xT[:, 0, :, :])
            pre_x[0] = xt_t
            wk_sb = wpool.tile([128, 6, INNER], bf16, tag="wk", name="wk")
            nc.scalar.dma_start(wk_sb[:], wk[:])
            ctx_tiles = {}
            for s in range(NSLOT):
                ctx_tiles[s] = ctxpool.tile([128, 6, S], bf16, tag=f"ctx{s}",
                                            name=f"ctx{s}")
            nc.sync.dma_start(ctx_tiles[0][:], ctxs[0][:])
            if CAP > 1:
                xt_t = mpool.tile([128, 2, TL], bf16, tag="x", name="x")
                nc.sync.dma_start(xt_t[:], xT[:, 1, :, :])
                pre_x[1] = xt_t
            wv_sb = wpool.tile([128, 6, INNER], bf16, tag="wv", name="wv")
            nc.sync.dma_start(wv_sb[:], wv[:])
            wo_sb = wpool.tile([128, 4, DQ], bf16, tag="wo", name="wo")
            nc.sync.dma_start(wo_sb[:], wo[:])
            bob_sb = wpool.tile([128, DQ], f32, tag="bob", name="bob")
            nc.sync.dma_start(bob_sb[:], bob[:])
            id_sb = wpool.tile([128, 128], bf16, tag="ident", name="ident")
            nc.sync.dma_start(id_sb[:], ident[:])
            for s in range(1, NSLOT):
                nc.sync.dma_start(ctx_tiles[s][:], ctxs[s][:])

            # ---- KV pieces: i in 0..3 -> kT m-chunk i; 4..7 -> v s-chunk i-4
            kT = {}
            vA = {}

            def kv_piece(s, i):
                ctx_sb = ctx_tiles[s]
                if s not in kT:
                    kT[s] = [kvpool.tile([128, S], bf16, tag=f"kT{s}_{m}",
                                         name=f"kT{s}_{m}") for m in range(4)]
                    vA[s] = [kvpool.tile([128, WVN], bf16, tag=f"v{s}_{sc}",
                                         name=f"v{s}_{sc}") for sc in range(4)]
                if i < 4:
                    m = i
                    pk = ps_big.tile([128, S], f32, tag="big", name="big")
                    for kc in range(6):
                        nc.tensor.matmul(
                            pk[:], wk_sb[:, kc, m * 128:(m + 1) * 128],
                            ctx_sb[:, kc, :], start=(kc == 0), stop=(kc == 5))
                    nc.vector.tensor_copy(kT[s][m][:], pk[:])
                else:
                    sc = i - 4
                    pv = ps_big.tile([128, 512], f32, tag="big", name="big")
                    for kc in range(6):
                        nc.tensor.matmul(
                            pv[:],
                            ctx_sb[:, kc, sc * 128:(sc + 1) * 128],
                            wv_sb[:, kc, :], start=(kc == 0), stop=(kc == 5))
                    vdst = vA[s][sc][:].rearrange("p (h d) -> p h d", d=65)
                    nc.vector.tensor_copy(
                        vdst[:, :, 0:DH],
                        pv[:].rearrange("p (h d) -> p h d", d=DH))
                    nc.gpsimd.memset(vdst[:, :, DH:65], 1.0)

            # ---- helpers for the main loop ----
            def emit_scores(qT_t, s, h):
                """4 score matmuls + 2 exps for head h; returns 4 expT
                slices [(etile, j)] for s-chunks 0..3."""
                c, half = h // 2, h % 2
                out = []
                for g in range(2):
                    psc = ps_sc.tile([128, 2, TL], f32, tag="sc", name="sc")
                    for j in range(2):
                        sc = g * 2 + j
                        nc.tensor.matmul(
                            psc[:, j, :],
                            kT[s][c][half * 64:(half + 1) * 64,
                                     sc * 128:(sc + 1) * 128],
                            qT_t[c][half * 64:(half + 1) * 64, :],
                            start=True, stop=True)
                    e = epool.tile([128, 2, TL], bf16, tag=f"e{g}", name=f"e{g}")
                    nc.scalar.activation(
                        e[:], psc[:], mybir.ActivationFunctionType.Exp)
                    out.extend([(e, 0), (e, 1)])
                return out

            def emit_av(s, grp, exps, avn_out, qcs=(0, 1, 2, 3)):
                """AV + normalize for heads grp*4..grp*4+3; fills
                avn_out[grp] = list of 4 avn tiles (one per q-chunk)."""
                for qc in qcs:
                    pav = ps_av.tile([128, 4, 65], f32, tag="av", name="av")
                    for hh in range(4):
                        h = grp * 4 + hh
                        for sc in range(4):
                            e, j = exps[h][sc]
                            nc.tensor.matmul(
                                pav[:, hh, :],
                                e[:, j, qc * 128:(qc + 1) * 128],
                                vA[s][sc][:, h * 65:(h + 1) * 65],
                                start=(sc == 0), stop=(sc == 3))
                    rcp = spool.tile([128, 4], f32, tag="rcp", name="rcp")
                    nc.vector.reciprocal(rcp[:], pav[:, :, 64:65])
                    avn = anpool.tile([128, 4, DH], bf16, tag="avn", name="avn")
                    nc.vector.tensor_mul(
                        avn[:], pav[:, :, 0:DH],
                        rcp[:].rearrange("p (h o) -> p h o", o=1)
                        .broadcast_to([128, 4, DH]))
                    avn_out[grp][qc] = avn

            def emit_transp(avn_tiles):
                """Transpose avn [q, inner] -> 4 sbuf tiles [128 i, 512 q]."""
                avT = []
                for ic in range(4):
                    tp = ps_big.tile([128, 512], bf16, tag="big", name="big")
                    grp, pr = ic // 2, ic % 2
                    for qc in range(4):
                        nc.tensor.transpose(
                            tp[:, qc * 128:(qc + 1) * 128],
                            avn_tiles[grp][qc][:, pr * 2:pr * 2 + 2, :],
                            id_sb[:])
                    av_sb = ovpool.tile([128, 512], bf16, tag="avT", name="avT")
                    nc.vector.tensor_copy(av_sb[:], tp[:])
                    avT.append(av_sb)
                return avT

            def emit_oproj(tt, avT):
                for pair in range(2):
                    yt = ypool.tile([128, 2, DQ], f32, tag="y", name="y")
                    for j in range(2):
                        lsub = pair * 2 + j
                        po = ps_big.tile([128, DQ], f32, tag="big", name="big")
                        for kc in range(4):
                            nc.tensor.matmul(
                                po[:], avT[kc][:, lsub * 128:(lsub + 1) * 128],
                                wo_sb[:, kc, :], start=(kc == 0), stop=(kc == 3))
                        nc.vector.tensor_add(yt[:, j, :], po[:], bob_sb[:])
                    ydst = y[tt * TL + pair * 256: tt * TL + (pair + 1) * 256, :]
                    nc.sync.dma_start(
                        ydst.rearrange("(j p) n -> p j n", j=2), yt[:])

            # ---- main loop over query tiles ----
            # pending work carried into the next tile (keeps PE fed while
            # ACT finishes the current tile's exps).
            # kv_sched[t] = [(slot, piece, sc_pos)]: emit KV piece right
            # before score head sc_pos of tile t, so the in-order PE stream
            # never runs a long KV burst that would starve ACT.
            pend = None  # (t, s, exps, avn_tiles)
            kv_sched = {0: [(0, 0, 0), (0, 1, 2), (0, 4, 3), (0, 2, 4),
                            (0, 5, 4), (0, 3, 5), (0, 6, 5), (0, 7, 5)]}
            if NSLOT > 1:
                free_tiles = list(range(1, cap_a))
                if len(free_tiles) >= 4:
                    sched = []
                    ft = free_tiles[-4:]
                    for j in range(8):
                        sched.append((1, j, 2 if j % 2 == 0 else 5))
                    kv_sched.update({
                        ft[0]: [sched[0], sched[1]],
                        ft[1]: [sched[2], sched[3]],
                        ft[2]: [sched[4], sched[5]],
                        ft[3]: [sched[6], sched[7]],
                    })
                elif free_tiles:
                    per = 8 // len(free_tiles) + 1
                    k = 0
                    for ftl in free_tiles:
                        pcs = []
                        for _ in range(per):
                            if k < 8:
                                pcs.append((1, k, 2 + 3 * (len(pcs) % 2)))
                                k += 1
                        kv_sched[ftl] = pcs
                    if k < 8:
                        kv_sched[0] = kv_sched[0] + [
                            (1, i, 5) for i in range(k, 8)]
                else:
                    kv_sched[0] = kv_sched[0] + [(1, i, 5) for i in range(8)]

            for t in range(CAP):
                s = 0 if t < cap_a else 1
                sched_t = kv_sched.get(t, [])

                def kv_at(pos):
                    for sl, i, p in sched_t:
                        if p == pos:
                            kv_piece(sl, i)

                if t in pre_x:
                    xt = pre_x.pop(t)
                else:
                    xt = mpool.tile([128, 2, TL], bf16, tag="x", name="x")
                    nc.sync.dma_start(xt[:], xT[:, t, :, :])
                qT_t = [qpool.tile([128, TL], bf16, tag=f"q{m}", name=f"q{m}") for m in range(4)]
                for m in range(4):
                    pq = ps_big.tile([128, TL], f32, tag="big", name="big")
                    for kc in range(2):
                        nc.tensor.matmul(
                            pq[:], wq_sb[:, kc, m * 128:(m + 1) * 128],
                            xt[:, kc, :], start=(kc == 0), stop=(kc == 1))
                    nc.vector.tensor_copy(qT_t[m][:], pq[:])

                exps = {}
                avn_tiles = {0: [None] * 4, 1: [None] * 4}
                kv_at(0)
                exps[0] = emit_scores(qT_t, s, 0)
                kv_at(1)
                exps[1] = emit_scores(qT_t, s, 1)
                if pend is not None:
                    pt, ps_, pexps, pavn = pend
                    emit_av(ps_, 1, pexps, pavn, qcs=(0, 1))
                kv_at(2)
                exps[2] = emit_scores(qT_t, s, 2)
                if pend is not None:
                    emit_av(ps_, 1, pexps, pavn, qcs=(2, 3))
                kv_at(3)
                exps[3] = emit_scores(qT_t, s, 3)
                if pend is not None:
                    pavT = emit_transp(pavn)
                kv_at(4)
                exps[4] = emit_scores(qT_t, s, 4)
                kv_at(5)
                exps[5] = emit_scores(qT_t, s, 5)
                if pend is not None:
                    emit_oproj(pt, pavT)
                    pend = None
                emit_av(s, 0, exps, avn_tiles, qcs=(0, 1))
                kv_at(6)
                exps[6] = emit_scores(qT_t, s, 6)
                emit_av(s, 0, exps, avn_tiles, qcs=(2, 3))
                kv_at(7)
                exps[7] = emit_scores(qT_t, s, 7)
                pend = (t, s, exps, avn_tiles)

            # flush the last tile's tail
            pt, ps_, pexps, pavn = pend
            emit_av(ps_, 1, pexps, pavn)
            pavT = emit_transp(pavn)
            emit_oproj(pt, pavT)
    nc.compile()
    return nc


def kernel(x, context, seq_lens, Wq, Wk, Wv, Wo, bo):
    from concourse.bass_utils import run_bass_kernel_spmd
    from ml_dtypes import bfloat16

    x = np.asarray(x, dtype=np.float32)
    context = np.asarray(context, dtype=np.float32)
    seq_lens = np.asarray(seq_lens, dtype=np.int32)
    Wq = np.asarray(Wq, dtype=np.float32)
    Wk = np.asarray(Wk, dtype=np.float32)
    Wv = np.asarray(Wv, dtype=np.float32)
    Wo = np.asarray(Wo, dtype=np.float32)
    bo = np.asarray(bo, dtype=np.float32)

    lens = np.clip(seq_lens, 1, L)
    nt = [int(math.ceil(int(n) / TL)) for n in lens]
    cap_a, cap_b, cores = _plan(nt)
    CAP = cap_a + cap_b
    NSLOT = 2 if cap_b > 0 else 1

    key = (cap_a, cap_b)
    if key not in _PROG_CACHE:
        _PROG_CACHE[key] = _build_program(cap_a, cap_b)
    nc = _PROG_CACHE[key]

    # shared (replicated) weights, packed as [128, chunks, free] so each
    # tensor is a single DMA
    def pack(w, nk):
        return np.ascontiguousarray(
            w.reshape(nk, 128, -1).transpose(1, 0, 2).astype(bfloat16))

    scale = 1.0 / math.sqrt(DH)
    wq_in = pack(Wq * scale, 2)
    wk_in = pack(Wk, 6)
    wv_in = pack(Wv, 6)
    wo_in = pack(Wo, 4)
    bob_in = np.broadcast_to(bo[None, :], (128, DQ)).copy()
    ident_in = np.eye(128, dtype=bfloat16)

    in_maps = []
    for core in range(N_CORES):
        xt_core = np.zeros((CAP * TL, DQ), dtype=np.float32)
        m = {}
        for sidx in range(NSLOT):
            bi, t0 = cores[core][sidx]
            npieces = cap_a if sidx == 0 else cap_b
            if bi >= 0:
                r0 = t0 * TL
                r1 = min(r0 + npieces * TL, L)
                if r1 > r0:
                    off = sidx * cap_a * TL
                    xt_core[off:off + (r1 - r0)] = x[bi, r0:r1]
                cb = context[bi]
            else:
                cb = context[0]
            m[f"ctx{sidx}"] = pack(cb.T, 6)
        m["xT"] = np.ascontiguousarray(
            xt_core.T.reshape(2, 128, CAP, TL).transpose(1, 2, 0, 3)
            .astype(bfloat16))
        m["wq"] = wq_in
        m["wk"] = wk_in
        m["wv"] = wv_in
        m["wo"] = wo_in
        m["bob"] = bob_in
        m["ident"] = ident_in
        in_maps.append(m)

    res = run_bass_kernel_spmd(nc, in_maps, list(range(N_CORES)))

    out = np.zeros((B, L, DQ), dtype=np.float32)
    for core in range(N_CORES):
        yc = res.results[core]["y"]
        for sidx in range(NSLOT):
            bi, t0 = cores[core][sidx]
            if bi < 0:
                continue
            npieces = cap_a if sidx == 0 else cap_b
            r0 = t0 * TL
            r1 = min(r0 + npieces * TL, int(lens[bi]))
            if r1 > r0:
                off = sidx * cap_a * TL
                out[bi, r0:r1] = yc[off:off + (r1 - r0)]
    return out


# revision 26
# speedup vs baseline: 1.0451x; 1.0254x over previous
"""CrossAttention Trainium2 kernel.

Reference computation (per batch b):
  q = x @ Wq; k = ctx @ Wk; v = ctx @ Wv   (multi-head, H=8, DH=64)
  out = softmax(q k^T / sqrt(DH)) v @ Wo + bo, rows >= seq_len zeroed.

Strategy: only rows < seq_len are computed ("ragged"); valid query tiles
(512 rows) are distributed across the 8 cores with a statically uniform
per-core structure: every core runs CAP query tiles, the first CAP_A of
which read KV slot A and the rest KV slot B. Which batch each slot holds
is per-core DATA (context tensors), so one SPMD program serves all cores.

Per query tile (Lt = 512 rows):
  xT [DQ, Lt] -> qT = Wq^T xT [INNER, Lt]            (f32r matmuls)
  kT = Wk^T ctxT [INNER, S] per slot                 (f32r)
  v_aug [S, 8, 65] = ctx @ Wv + ones col per head    (bf16)
  scoresT_h [S, Lt] = kT_h^T qT_h  (per head, K=DH)  (f32r)
  expT = exp(scoresT) -> bf16  (no max subtraction: logits ~ N(0,1))
  AV reoriented: av[q, d]: out [128q, 4h, 65] psum, lhsT = expT chunk
    (stationary), rhs = v_aug_h [s, 65] moving, N=65 bf16 (1 cyc/row) --
    2.3x fewer PE cycles than the avT orientation, and the softmax
    denominator (col 64) lands on the q-PARTITION axis where
    normalization is cheap:
  norm: rcp[q, h] = 1/pav[:, :, 64];  avn = pav[:, :, 0:64] * rcp  (DVE)
  transpose avn -> avT [INNER, Lt] via PE is_transpose matmuls (bf16)
  out [Lt, DQ] = avT^T @ Wo + bob                     (bf16 matmuls)
Emission interleaves next-tile scores with the current tile's AV /
transpose / output projection so the in-order PE and ACT streams both
stay busy; slot-B KV matmuls are spread across the two tiles preceding
the slot boundary.
"""

import math
import sys

sys.path.insert(0, "/opt/trn_rl_repo")

import numpy as np

B, L, S = 8, 8192, 512
DQ, DC = 256, 768
H, DH = 8, 64
INNER = H * DH
TL = 512          # query rows per tile
N_CORES = 8
WVN = H * 65      # 520: v augmented with a ones col per head


def _plan(nt):
    """Choose slot sizes (1-3 slots) and per-core pieces. Returns
    (caps, cores): caps = tuple of per-slot tile counts summing to CAP;
    cores[i] = list of (batch, tile0) per slot (batch -1 = padding).
    Cost model: CAP * 17 (per-tile work) + NSLOT * 10.5 (KV phases)."""
    import itertools
    best = None
    lo = max(1, math.ceil(sum(nt) / N_CORES))
    hi = max(max(nt), lo) + 2
    for cap in range(lo, hi):
        cands = [(cap,)]
        for b in range(1, cap):
            if cap - b >= b:
                cands.append((cap - b, b))
        for b in range(1, cap):
            for c in range(1, b + 1):
                if cap - b - c >= b:
                    cands.append((cap - b - c, b, c))
        for caps in cands:
            ns = len(caps)
            cost = cap * 17.0 + ns * 10.5
            if best is not None and cost >= best[0]:
                continue

            def options(n):
                outs = []
                for ks in itertools.product(*([range(0, N_CORES + 1)] * ns)):
                    cov = sum(k * c for k, c in zip(ks, caps))
                    if cov < n:
                        continue
                    if any(ks[i] and cov - caps[i] >= n for i in range(ns)):
                        continue
                    outs.append(ks)
                outs.sort(key=lambda ks: sum(k * c for k, c in zip(ks, caps)))
                return outs[:24]

            opts = [options(n) for n in nt]
            found = None

            def dfs(i, rem, acc):
                nonlocal found
                if found:
                    return
                if i == len(nt):
                    found = list(acc)
                    return
                for ks in opts[i]:
                    if all(ks[j] <= rem[j] for j in range(ns)):
                        acc.append(ks)
                        dfs(i + 1, [rem[j] - ks[j] for j in range(ns)], acc)
                        acc.pop()
                        if found:
                            return

            dfs(0, [N_CORES] * ns, [])
            if found:
                best = (cost, caps, found)
    assert best is not None
    _, caps, ks_all = best
    slots = [[] for _ in caps]
    for bi, ks in enumerate(ks_all):
        t = 0
        for s, k in enumerate(ks):
            for _ in range(k):
                slots[s].append((bi, t))
                t += caps[s]
    cores = []
    for i in range(N_CORES):
        cores.append([slots[s][i] if i < len(slots[s]) else (-1, 0)
                      for s in range(len(caps))])
    return caps, cores


_PROG_CACHE = {}


def _build_program(caps):
    import concourse.mybir as mybir
    import concourse.tile as tile
    from concourse import bacc

    f32 = mybir.dt.float32
    f32r = mybir.dt.float32r
    bf16 = mybir.dt.bfloat16
    CAP = sum(caps)
    NSLOT = len(caps)
    start = [sum(caps[:s]) for s in range(NSLOT)]

    def slot_of(t):
        for s in range(NSLOT - 1, -1, -1):
            if t >= start[s]:
                return s
        return 0

    nc = bacc.Bacc("TRN2", target_bir_lowering=False, debug=False,
                   num_devices=N_CORES)
    xT = nc.declare_dram_parameter("xT", [128, CAP, 2, TL], bf16, isOutput=False)
    ctxs = [nc.declare_dram_parameter(f"ctx{s}", [128, 6, S], bf16, isOutput=False)
            for s in range(NSLOT)]
    wq = nc.declare_dram_parameter("wq", [128, 2, INNER], bf16, isOutput=False)
    wk = nc.declare_dram_parameter("wk", [128, 6, INNER], bf16, isOutput=False)
    wv = nc.declare_dram_parameter("wv", [128, 6, INNER], bf16, isOutput=False)
    wo = nc.declare_dram_parameter("wo", [128, 4, DQ], bf16, isOutput=False)
    bob = nc.declare_dram_parameter("bob", [128, DQ], f32, isOutput=False)
    ident = nc.declare_dram_parameter("ident", [128, 128], bf16, isOutput=False)
    y = nc.declare_dram_parameter("y", [CAP * TL, DQ], f32, isOutput=True)

    with tile.TileContext(nc) as tc:
        with (
            tc.tile_pool(name="wpool", bufs=1) as wpool,
            tc.tile_pool(name="kvpool", bufs=1) as kvpool,
            tc.tile_pool(name="ctxpool", bufs=1) as ctxpool,
            tc.tile_pool(name="mpool", bufs=4) as mpool,
            tc.tile_pool(name="qpool", bufs=3) as qpool,
            tc.tile_pool(name="epool", bufs=14) as epool,
            tc.tile_pool(name="anpool", bufs=12) as anpool,
            tc.tile_pool(name="ovpool", bufs=6) as ovpool,
            tc.tile_pool(name="spool", bufs=4) as spool,
            tc.tile_pool(name="ypool", bufs=4) as ypool,
            tc.tile_pool(name="ps_big", bufs=2, space="PSUM") as ps_big,
            tc.tile_pool(name="ps_sc", bufs=2, space="PSUM") as ps_sc,
            tc.tile_pool(name="ps_av", bufs=2, space="PSUM") as ps_av,
        ):
            # ---- load weights. The head is DMA-latency critical: spread
            # the issue cost across idle engine queues (SP / ACT / DVE all
            # have their own DGE rings) so the shared DMA engines are the
            # only serializer before the first score matmul.
            wq_sb = wpool.tile([128, 2, INNER], bf16, tag="wq", name="wq")
            nc.sync.dma_start(wq_sb[:], wq[:])
            pre_x = {}
            xt_t = mpool.tile([128, 2, TL], bf16, tag="x", name="x")
            nc.sync.dma_start(xt_t[:], xT[:, 0, :, :])
            pre_x[0] = xt_t
            wk_sb = wpool.tile([128, 6, INNER], bf16, tag="wk", name="wk")
            nc.scalar.dma_start(wk_sb[:], wk[:])
            ctx_tiles = {}
            for s in range(NSLOT):
                ctx_tiles[s] = ctxpool.tile([128, 6, S], bf16, tag=f"ctx{s}",
                                            name=f"ctx{s}")
            nc.sync.dma_start(ctx_tiles[0][:], ctxs[0][:])
            if CAP > 1:
                xt_t = mpool.tile([128, 2, TL], bf16, tag="x", name="x")
                nc.sync.dma_start(xt_t[:], xT[:, 1, :, :])
                pre_x[1] = xt_t
            wv_sb = wpool.tile([128, 6, INNER], bf16, tag="wv", name="wv")
            nc.sync.dma_start(wv_sb[:], wv[:])
            wo_sb = wpool.tile([128, 4, DQ], bf16, tag="wo", name="wo")
            nc.sync.dma_start(wo_sb[:], wo[:])
            bob_sb = wpool.tile([128, DQ], f32, tag="bob", name="bob")
            nc.sync.dma_start(bob_sb[:], bob[:])
            id_sb = wpool.tile([128, 128], bf16, tag="ident", name="ident")
            nc.sync.dma_start(id_sb[:], ident[:])
            for s in range(1, NSLOT):
                nc.sync.dma_start(ctx_tiles[s][:], ctxs[s][:])

            # ---- KV pieces: i in 0..3 -> kT m-chunk i; 4..7 -> v s-chunk i-4
            kT = {}
            vA = {}

            def kv_piece(s, i):
                ctx_sb = ctx_tiles[s]
                if s not in kT:
                    kT[s] = [kvpool.tile([128, S], bf16, tag=f"kT{s}_{m}",
                                         name=f"kT{s}_{m}") for m in range(4)]
                    vA[s] = [kvpool.tile([128, WVN], bf16, tag=f"v{s}_{sc}",
                                         name=f"v{s}_{sc}") for sc in range(4)]
                if i < 4:
                    m = i
                    pk = ps_big.tile([128, S], f32, tag="big", name="big")
                    for kc in range(6):
                        nc.tensor.matmul(
                            pk[:], wk_sb[:, kc, m * 128:(m + 1) * 128],
                            ctx_sb[:, kc, :], start=(kc == 0), stop=(kc == 5))
                    nc.vector.tensor_copy(kT[s][m][:], pk[:])
                else:
                    sc = i - 4
                    pv = ps_big.tile([128, 512], f32, tag="big", name="big")
                    for kc in range(6):
                        nc.tensor.matmul(
                            pv[:],
                            ctx_sb[:, kc, sc * 128:(sc + 1) * 128],
                            wv_sb[:, kc, :], start=(kc == 0), stop=(kc == 5))
                    vdst = vA[s][sc][:].rearrange("p (h d) -> p h d", d=65)
                    nc.vector.tensor_copy(
                        vdst[:, :, 0:DH],
                        pv[:].rearrange("p (h d) -> p h d", d=DH))
                    nc.gpsimd.memset(vdst[:, :, DH:65], 1.0)

            # ---- helpers for the main loop ----
            def emit_scores(qT_t, s, h):
                """4 score matmuls + 2 exps for head h; returns 4 expT
                slices [(etile, j)] for s-chunks 0..3."""
                c, half = h // 2, h % 2
                out = []
                for g in range(2):
                    psc = ps_sc.tile([128, 2, TL], f32, tag="sc", name="sc")
                    for j in range(2):
                        sc = g * 2 + j
                        nc.tensor.matmul(
                            psc[:, j, :],
                            kT[s][c][half * 64:(half + 1) * 64,
                                     sc * 128:(sc + 1) * 128],
                            qT_t[c][half * 64:(half + 1) * 64, :],
                            start=True, stop=True)
                    e = epool.tile([128, 2, TL], bf16, tag=f"e{g}", name=f"e{g}")
                    nc.scalar.activation(
                        e[:], psc[:], mybir.ActivationFunctionType.Exp)
                    out.extend([(e, 0), (e, 1)])
                return out

            def emit_av(s, grp, exps, avn_out, qcs=(0, 1, 2, 3)):
                """AV + normalize for heads grp*4..grp*4+3; fills
                avn_out[grp] = list of 4 avn tiles (one per q-chunk)."""
                for qc in qcs:
                    pav = ps_av.tile([128, 4, 65], f32, tag="av", name="av")
                    for hh in range(4):
                        h = grp * 4 + hh
                        for sc in range(4):
                            e, j = exps[h][sc]
                            nc.tensor.matmul(
                                pav[:, hh, :],
                                e[:, j, qc * 128:(qc + 1) * 128],
                                vA[s][sc][:, h * 65:(h + 1) * 65],
                                start=(sc == 0), stop=(sc == 3))
                    rcp = spool.tile([128, 4], f32, tag="rcp", name="rcp")
                    nc.vector.reciprocal(rcp[:], pav[:, :, 64:65])
                    avn = anpool.tile([128, 4, DH], bf16, tag="avn", name="avn")
                    nc.vector.tensor_mul(
                        avn[:], pav[:, :, 0:DH],
                        rcp[:].rearrange("p (h o) -> p h o", o=1)
                        .broadcast_to([128, 4, DH]))
                    avn_out[grp][qc] = avn

            def emit_transp(avn_tiles):
                """Transpose avn [q, inner] -> 4 sbuf tiles [128 i, 512 q]."""
                avT = []
                for ic in range(4):
                    tp = ps_big.tile([128, 512], bf16, tag="big", name="big")
                    grp, pr = ic // 2, ic % 2
                    for qc in range(4):
                        nc.tensor.transpose(
                            tp[:, qc * 128:(qc + 1) * 128],
                            avn_tiles[grp][qc][:, pr * 2:pr * 2 + 2, :],
                            id_sb[:])
                    av_sb = ovpool.tile([128, 512], bf16, tag="avT", name="avT")
                    nc.vector.tensor_copy(av_sb[:], tp[:])
                    avT.append(av_sb)
                return avT

            def emit_oproj(tt, avT):
                for pair in range(2):
                    yt = ypool.tile([128, 2, DQ], f32, tag="y", name="y")
                    for j in range(2):
                        lsub = pair * 2 + j
                        po = ps_big.tile([128, DQ], f32, tag="big", name="big")
                        for kc in range(4):
                            nc.tensor.matmul(
                                po[:], avT[kc][:, lsub * 128:(lsub + 1) * 128],
                                wo_sb[:, kc, :], start=(kc == 0), stop=(kc == 3))
                        nc.vector.tensor_add(yt[:, j, :], po[:], bob_sb[:])
                    ydst = y[tt * TL + pair * 256: tt * TL + (pair + 1) * 256, :]
                    nc.sync.dma_start(
                        ydst.rearrange("(j p) n -> p j n", j=2), yt[:])

            # ---- main loop over query tiles ----
            # pending work carried into the next tile (keeps PE fed while
            # ACT finishes the current tile's exps).
            # kv_sched[t] = [(slot, piece, sc_pos)]: emit KV piece right
            # before score head sc_pos of tile t, so the in-order PE stream
            # never runs a long KV burst that would starve ACT.
            pend = None  # (t, s, exps, avn_tiles)
            # slot 0: kT piece p right before head pair p (tile 0), v
            # pieces late but before the first AV group
            kv_sched = {0: [(0, 0, 0), (0, 1, 2), (0, 4, 3), (0, 2, 4),
                            (0, 5, 4), (0, 3, 5), (0, 6, 5), (0, 7, 5)]}
            # slots >= 1: greedy by deadline, <=3 pieces per tile, emitted
            # at score positions 1 / 4 / 7
            cap_per_tile = 3
            load = {}
            for s in range(1, NSLOT):
                for i in range(8):
                    # latest allowed tile is start[s] - 1
                    tt = start[s] - 1
                    while tt > 0 and load.get(tt, 0) >= cap_per_tile:
                        tt -= 1
                    if tt <= 0:
                        tt = max(1, start[s] - 1)  # overflow: stack anyway
                    pos = (1, 4, 7)[load.get(tt, 0) % 3]
                    kv_sched.setdefault(tt, []).append((s, i, pos))
                    load[tt] = load.get(tt, 0) + 1

            for t in range(CAP):
                s = slot_of(t)
                sched_t = kv_sched.get(t, [])

                def kv_at(pos):
                    for sl, i, p in sched_t:
                        if p == pos:
                            kv_piece(sl, i)

                if t in pre_x:
                    xt = pre_x.pop(t)
                else:
                    xt = mpool.tile([128, 2, TL], bf16, tag="x", name="x")
                    nc.sync.dma_start(xt[:], xT[:, t, :, :])
                qT_t = [qpool.tile([128, TL], bf16, tag=f"q{m}", name=f"q{m}") for m in range(4)]

                def emit_qT(m):
                    pq = ps_big.tile([128, TL], f32, tag="big", name="big")
                    for kc in range(2):
                        nc.tensor.matmul(
                            pq[:], wq_sb[:, kc, m * 128:(m + 1) * 128],
                            xt[:, kc, :], start=(kc == 0), stop=(kc == 1))
                    nc.vector.tensor_copy(qT_t[m][:], pq[:])

                exps = {}
                avn_tiles = {0: [None] * 4, 1: [None] * 4}
                emit_qT(0)
                kv_at(0)
                exps[0] = emit_scores(qT_t, s, 0)
                kv_at(1)
                exps[1] = emit_scores(qT_t, s, 1)
                if pend is not None:
                    pt, ps_, pexps, pavn = pend
                    emit_av(ps_, 1, pexps, pavn, qcs=(0, 1))
                emit_qT(1)
                kv_at(2)
                exps[2] = emit_scores(qT_t, s, 2)
                if pend is not None:
                    emit_av(ps_, 1, pexps, pavn, qcs=(2, 3))
                kv_at(3)
                exps[3] = emit_scores(qT_t, s, 3)
                if pend is not None:
                    pavT = emit_transp(pavn)
                emit_qT(2)
                kv_at(4)
                exps[4] = emit_scores(qT_t, s, 4)
                kv_at(5)
                exps[5] = emit_scores(qT_t, s, 5)
                if pend is not None:
                    emit_oproj(pt, pavT)
                    pend = None
                emit_av(s, 0, exps, avn_tiles, qcs=(0, 1))
                emit_qT(3)
                kv_at(6)
                exps[6] = emit_scores(qT_t, s, 6)
                emit_av(s, 0, exps, avn_tiles, qcs=(2, 3))
                kv_at(7)
                exps[7] = emit_scores(qT_t, s, 7)
                pend = (t, s, exps, avn_tiles)

            # flush the last tile's tail
            pt, ps_, pexps, pavn = pend
            emit_av(ps_, 1, pexps, pavn)
            pavT = emit_transp(pavn)
            emit_oproj(pt, pavT)
    nc.compile()
    return nc


def kernel(x, context, seq_lens, Wq, Wk, Wv, Wo, bo):
    from concourse.bass_utils import run_bass_kernel_spmd
    from ml_dtypes import bfloat16

    x = np.asarray(x, dtype=np.float32)
    context = np.asarray(context, dtype=np.float32)
    seq_lens = np.asarray(seq_lens, dtype=np.int32)
    Wq = np.asarray(Wq, dtype=np.float32)
    Wk = np.asarray(Wk, dtype=np.float32)
    Wv = np.asarray(Wv, dtype=np.float32)
    Wo = np.asarray(Wo, dtype=np.float32)
    bo = np.asarray(bo, dtype=np.float32)

    lens = np.clip(seq_lens, 1, L)
    nt = [int(math.ceil(int(n) / TL)) for n in lens]
    caps, cores = _plan(nt)
    CAP = sum(caps)
    NSLOT = len(caps)
    start = [sum(caps[:s]) for s in range(NSLOT)]

    key = tuple(caps)
    if key not in _PROG_CACHE:
        _PROG_CACHE[key] = _build_program(caps)
    nc = _PROG_CACHE[key]

    # shared (replicated) weights, packed as [128, chunks, free] so each
    # tensor is a single DMA
    def pack(w, nk):
        return np.ascontiguousarray(
            w.reshape(nk, 128, -1).transpose(1, 0, 2).astype(bfloat16))

    scale = 1.0 / math.sqrt(DH)
    wq_in = pack(Wq * scale, 2)
    wk_in = pack(Wk, 6)
    wv_in = pack(Wv, 6)
    wo_in = pack(Wo, 4)
    bob_in = np.broadcast_to(bo[None, :], (128, DQ)).copy()
    ident_in = np.eye(128, dtype=bfloat16)

    in_maps = []
    for core in range(N_CORES):
        xt_core = np.zeros((CAP * TL, DQ), dtype=np.float32)
        m = {}
        for sidx in range(NSLOT):
            bi, t0 = cores[core][sidx]
            npieces = caps[sidx]
            if bi >= 0:
                r0 = t0 * TL
                r1 = min(r0 + npieces * TL, L)
                if r1 > r0:
                    off = start[sidx] * TL
                    xt_core[off:off + (r1 - r0)] = x[bi, r0:r1]
                cb = context[bi]
            else:
                cb = context[0]
            m[f"ctx{sidx}"] = pack(cb.T, 6)
        m["xT"] = np.ascontiguousarray(
            xt_core.T.reshape(2, 128, CAP, TL).transpose(1, 2, 0, 3)
            .astype(bfloat16))
        m["wq"] = wq_in
        m["wk"] = wk_in
        m["wv"] = wv_in
        m["wo"] = wo_in
        m["bob"] = bob_in
        m["ident"] = ident_in
        in_maps.append(m)

    res = run_bass_kernel_spmd(nc, in_maps, list(range(N_CORES)))

    out = np.zeros((B, L, DQ), dtype=np.float32)
    for core in range(N_CORES):
        yc = res.results[core]["y"]
        for sidx in range(NSLOT):
            bi, t0 = cores[core][sidx]
            if bi < 0:
                continue
            npieces = caps[sidx]
            r0 = t0 * TL
            r1 = min(r0 + npieces * TL, int(lens[bi]))
            if r1 > r0:
                off = start[sidx] * TL
                out[bi, r0:r1] = yc[off:off + (r1 - r0)]
    return out
